# revision 16
# baseline (speedup 1.0000x reference)
"""Causal multi-head attention (B=2, S=2048, D=1024, H=16) on 8 trn2 cores.

Sharding: core c handles heads {2c, 2c+1} of BOTH batches (4 (b,h) pairs).
All matmul inputs are bf16 (host-rounded); accumulation stays fp32 in PSUM.

Per core:
  - project host-pretransposed x_b^T [D, S] (both batches) through the
    core's Wqkv column slice into Q^T/K^T head-pair tiles (bf16).  V is
    produced transposed (long moving dim), then flipped to natural layout
    with PE transposes; a fused ones-column makes AV emit softmax
    denominators,
  - causal attention per (batch, head-pair) in transposed layout:
    scores^T = K Q^T chunks as two row-tiled (tile_position) matmuls that
    stream concurrently, exp on ScalarE (bf16 out), diagonal masks via
    affine_select, A^T V accumulation on PE,
  - the head exchange is FOUR quarter-wise 8-way AllToAlls, fired as each
    sequence quarter finishes attention, so they overlap later attention.
    Sub-A2A q block t = (my heads, batch t//4, quarter q, col-slice t%4),
    so core i ends up owning tokens {512q + 128*(i%4)} of batch i//4 with
    ALL heads, and runs the output projection per received 128-token chunk.
    A dummy AllToAll issued at program start absorbs the ~11us collective
    firmware warmup.
Host assembles the 8x4 [128, 1024] shards into (2, 2048, 1024).
"""

import sys

for _p in ("/opt/trn_rl_repo", "/opt/pypackages"):
    if _p not in sys.path:
        sys.path.insert(0, _p)

import numpy as np
import ml_dtypes

import concourse.bass as bass
import concourse.mybir as mybir
import concourse.tile as tile
from concourse import bacc
from concourse.bass_utils import run_bass_kernel_spmd

B = 2
S = 2048
D = 1024
H = 16
DH = 64
NCORES = 8
SB = 512           # q block (matmul moving dim)
KC = 128           # k chunk (contraction tile)
NSB = S // SB      # 4 q-blocks
NKC = S // KC      # 16 k-chunks
NDC = D // KC      # 8 contraction chunks for the projections

_compiled = None


def _build():
    f32 = mybir.dt.float32
    bf16 = mybir.dt.bfloat16
    nc = bacc.Bacc(None, target_bir_lowering=False)

    # host-blocked inputs (bf16): xt[b, s, k] = x_b^T[128k:128k+128, 512s:+512]
    xt = nc.declare_dram_parameter("xt", [B, NSB, NDC, KC, SB], bf16, isOutput=False)
    # wqk cols: Q_ha | Q_hb | K_ha | K_hb (64 each)
    wqk = nc.declare_dram_parameter("wqk", [NDC, KC, 2 * KC], bf16, isOutput=False)
    # wv cols: V_ha | V_hb
    wv = nc.declare_dram_parameter("wv", [NDC, KC, KC], bf16, isOutput=False)
    wout = nc.declare_dram_parameter("wout", [NDC, KC, D], bf16, isOutput=False)
    bqk = nc.declare_dram_parameter("bqk", [2 * KC], f32, isOutput=False)
    bv = nc.declare_dram_parameter("bv", [2 * DH], f32, isOutput=False)
    bo = nc.declare_dram_parameter("bo", [D], f32, isOutput=False)
    vones = nc.declare_dram_parameter("vones", [KC, NKC], bf16, isOutput=False)
    ident = nc.declare_dram_parameter("ident", [KC, KC], bf16, isOutput=False)
    # out[q] = final rows for tokens [512q + 128*(c%4), +128) of batch c//4
    out_ext = nc.declare_dram_parameter("out", [NSB, KC, D], f32, isOutput=True)

    # quarter-wise AllToAll staging: sub-A2A q block t =
    #   (my 128 head rows, batch t//4, quarter q, col-slice 128*(t%4))
    a2a_in = [nc.dram_tensor(f"a2a_in{q}", [NCORES, KC, KC], bf16)
              for q in range(NSB)]
    a2a_out = [nc.dram_tensor(f"a2a_out{q}", [NCORES, KC, KC], bf16)
               for q in range(NSB)]

    with tile.TileContext(nc) as tc:
        with (
            tc.tile_pool(name="qkv", bufs=1) as qkvp,
            tc.tile_pool(name="obuf", bufs=1) as op,
            tc.tile_pool(name="misc", bufs=1) as mp,
            tc.tile_pool(name="pbuf", bufs=1) as pb,
            tc.tile_pool(name="evict", bufs=1) as ep,
            tc.tile_pool(name="wpool", bufs=1) as wp,
            tc.tile_pool(name="xbuf", bufs=10) as xp,
            tc.tile_pool(name="recvp", bufs=1) as rp,
            tc.tile_pool(name="psum", bufs=1, space="PSUM") as pp,
        ):
            # ---- weights + small constants --------------------------------
            # sync queue: wqk first (needed by the very first matmul), then
            # the first x tiles.  Big wout load rides the vector queue.
            wqk_t = wp.tile([KC, NDC * 2 * KC], bf16, tag="wqk")
            nc.sync.dma_start(
                out=wqk_t[:].rearrange("p (k c) -> p k c", k=NDC),
                in_=wqk.rearrange("k p c -> p k c"))
            bqk_t = [mp.tile([KC, 1], f32, tag=f"bqk{m}", name=f"bqk{m}")
                     for m in range(2)]
            for m in range(2):
                nc.scalar.dma_start(
                    out=bqk_t[m][:],
                    in_=bqk[m * KC:(m + 1) * KC].rearrange("(p o) -> p o", o=1),
                )
            bv_t = mp.tile([KC, 1], f32, tag="bv_t")
            nc.scalar.dma_start(
                out=bv_t[:], in_=bv.rearrange("(p o) -> p o", o=1))
            ident_t = mp.tile([KC, KC], bf16, tag="ident")
            nc.scalar.dma_start(out=ident_t[:], in_=ident[:])
            vones_sb = mp.tile([KC, NKC], bf16, tag="vones_sb")
            nc.scalar.dma_start(out=vones_sb[:], in_=vones[:])
            act_warm = mp.tile([1, 2], f32, tag="act_warm")
            nc.scalar.activation(
                act_warm[:], vones_sb[0:1, 0:2],
                mybir.ActivationFunctionType.Exp, scale=1.0)
            wv_t = wp.tile([KC, NDC * KC], bf16, tag="wv")
            nc.scalar.dma_start(
                out=wv_t[:].rearrange("p (k c) -> p k c", k=NDC),
                in_=wv.rearrange("k p c -> p k c"))
            wout_t = wp.tile([KC, NDC * D], bf16, tag="wout")
            bo_bc_box = []

            def load_bo():
                nc.sync.dma_start(
                    out=wout_t[:].rearrange("p (k c) -> p k c", k=NDC),
                    in_=wout.rearrange("k p c -> p k c"))
                bo_row = mp.tile([1, D], f32, tag="bo_row")
                nc.scalar.dma_start(
                    out=bo_row[:], in_=bo.rearrange("(o f) -> o f", o=1))
                t = mp.tile([KC, D], f32, tag="bo_bc")
                nc.gpsimd.partition_broadcast(out_ap=t[:], in_ap=bo_row[:])
                bo_bc_box.append(t)

            # ---- persistent activations -----------------------------------
            # QQ[p][s]: rows 0:64 = Q^T head 2c, 64:128 = head 2c+1 (batch p)
            QQ = [[qkvp.tile([KC, SB], bf16, tag=f"QQ{p}_{s}", name=f"QQ{p}_{s}")
                   for s in range(NSB)] for p in range(B)]
            KK = [[qkvp.tile([KC, SB], bf16, tag=f"KK{p}_{s}", name=f"KK{p}_{s}")
                   for s in range(NSB)] for p in range(B)]
            # V[2p+hh][s]: [128, 4*65]; chunk sc at cols 65sc..+64, col 65sc+64=1
            NCS = SB // KC
            V = [[qkvp.tile([KC, NCS * (DH + 1)], bf16, tag=f"V{v}_{s}",
                            name=f"V{v}_{s}")
                  for s in range(NSB)] for v in range(2 * B)]
            for v in range(2 * B):
                for s in range(NSB):
                    vv = V[v][s][:].rearrange("p (k c) -> p k c", c=DH + 1)
                    nc.vector.tensor_copy(
                        vv[:, :, DH], vones_sb[:, s * NCS:(s + 1) * NCS])
            # O[p]: rows 0:64 = head 2c out^T (normalized), 64:128 = head 2c+1
            O = [op.tile([KC, S], bf16, tag=f"O{p}", name=f"O{p}")
                 for p in range(B)]

            # P score tiles (bf16, post-exp).  Tag per k-chunk; low chunks
            # are double-buffered since they recur every quarter.
            def p_tile(p, q, kc):
                return pb.tile([KC, 2, SB], bf16, tag=f"P{kc}",
                               name=f"P{p}_{q}_{kc}",
                               bufs=(2 if kc < 8 else 1))

            # ---------------------------------------------------------------
            def proj(sblk):
                vts = []
                for bb in range(B):
                    xs = []
                    for j in range(NDC // 2):
                        xtl = xp.tile([KC, 2 * SB], bf16, tag="xt")
                        if sblk == 0:
                            eng = (nc.sync, nc.scalar, nc.sync, nc.scalar)[j]
                        else:
                            eng = (nc.sync, nc.gpsimd, nc.sync, nc.gpsimd)[j]
                        eng.dma_start(
                            out=xtl[:].rearrange("p (k t) -> p k t", k=2),
                            in_=xt[bb, sblk, 2 * j:2 * j + 2].rearrange(
                                "k p t -> p k t"),
                        )
                        xs.append(xtl)

                    def xchunk(k):
                        return xs[k // 2][:, (k % 2) * SB:(k % 2 + 1) * SB]

                    # m = 0 -> Q^T pair, m = 1 -> K^T pair
                    for m in range(2):
                        ps = pp.tile([KC, SB], f32, tag="ps_qk", bufs=1)
                        for k in range(NDC):
                            nc.tensor.matmul(
                                ps[:],
                                wqk_t[:, (2 * k + m) * KC:(2 * k + m + 1) * KC],
                                xchunk(k),
                                start=(k == 0),
                                stop=(k == NDC - 1),
                            )
                        dest = (QQ if m == 0 else KK)[bb][sblk]
                        nc.vector.tensor_scalar_add(dest[:], ps[:], bqk_t[m][:])
                    # V^T: long moving dim, then flip via PE transposes below
                    ps = pp.tile([KC, SB], f32, tag="ps_qk", bufs=1)
                    for k in range(NDC):
                        nc.tensor.matmul(
                            ps[:],
                            wv_t[:, k * KC:(k + 1) * KC],
                            xchunk(k),
                            start=(k == 0),
                            stop=(k == NDC - 1),
                        )
                    vt = ep.tile([KC, SB], bf16, tag="vt", bufs=2,
                                 name=f"vt{bb}_{sblk}")
                    nc.vector.tensor_scalar_add(vt[:], ps[:], bv_t[:])
                    vts.append(vt)
                for bb in range(B):
                    pst = pp.tile([KC, SB], bf16, tag="ps_tr", bufs=1)
                    for sc in range(NCS):
                        nc.tensor.transpose(
                            pst[:, sc * KC:(sc + 1) * KC],
                            vts[bb][:, sc * KC:(sc + 1) * KC],
                            ident_t[:],
                        )
                    ps4 = pst[:].rearrange("p (k h c) -> p k h c", k=NCS, h=2)
                    for hh in range(2):
                        nc.vector.tensor_copy(
                            V[2 * bb + hh][sblk][:].rearrange(
                                "p (k c) -> p k c", c=DH + 1)[:, :, 0:DH],
                            ps4[:, :, hh, :],
                        )

            # ---------------------------------------------------------------
            def fire_collective(qblk):
                nc.gpsimd.collective_compute(
                    "AllToAll",
                    mybir.AluOpType.bypass,
                    replica_groups=[[0, 1, 2, 3, 4, 5, 6, 7]],
                    ins=[a2a_in[qblk][:]],
                    outs=[a2a_out[qblk][:]],
                )

            def attention(qblk, fire=True):
                nkc = 4 * (qblk + 1)
                for p in range(B):
                    pos = [pp.tile([DH + 1, SB], f32, tag=f"ps_av{hh}",
                                   bufs=1, name=f"po{hh}_{p}_{qblk}")
                           for hh in range(2)]
                    P = [None] * nkc

                    def scores(kc):
                        d = kc - 4 * qblk
                        c0 = KC * max(d, 0)
                        ps = pp.tile([KC, 2, SB], f32, tag="ps_s", bufs=2)
                        for hh in range(2):  # row-tiled, stream concurrently
                            r0 = hh * DH
                            nc.tensor.matmul(
                                ps[:, hh, c0:SB],
                                KK[p][kc // 4][r0:r0 + DH,
                                               (kc % 4) * KC:(kc % 4 + 1) * KC],
                                QQ[p][qblk][r0:r0 + DH, c0:SB],
                                start=True,
                                stop=True,
                            )
                        P[kc] = p_tile(p, qblk, kc)
                        nc.scalar.activation(
                            P[kc][:, :, c0:SB],
                            ps[:, :, c0:SB],
                            mybir.ActivationFunctionType.Exp,
                            scale=1.0 / float(np.sqrt(DH)),
                        )
                        if d >= 0:  # diagonal chunk: zero where k > q
                            nc.gpsimd.affine_select(
                                out=P[kc][:, :, c0:SB],
                                in_=P[kc][:, :, c0:SB],
                                pattern=[[0, 2], [1, SB - c0]],
                                compare_op=mybir.AluOpType.is_ge,
                                fill=0.0,
                                base=0,
                                channel_multiplier=-1,
                            )

                    def av(kc):
                        d = kc - 4 * qblk
                        c0 = KC * max(d, 0)
                        for hh in range(2):
                            nc.tensor.matmul(
                                pos[hh][:, c0:SB],
                                V[2 * p + hh][kc // 4][:,
                                    (kc % 4) * (DH + 1):
                                    (kc % 4 + 1) * (DH + 1)],
                                P[kc][:, hh, c0:SB],
                                start=(kc == 0),
                                stop=(kc == nkc - 1),
                            )

                    # interleave: sc(kc) | av(kc-1) keeps ScalarE saturated
                    for kc in range(nkc):
                        scores(kc)
                        if kc >= 1:
                            av(kc - 1)
                    av(nkc - 1)

                    # normalize (PSUM reads stay on DVE; broadcast on GpSimd).
                    # For quarters 0-2, evacuate pos to SBUF in one copy so
                    # the next batch's AV can reuse the PSUM bank at once;
                    # for the tail quarter read PSUM directly (shorter chain).
                    if qblk < 3:
                        avst = [ep.tile([DH + 1, SB], f32, tag=f"avst{hh}",
                                        bufs=2, name=f"avst{hh}_{p}_{qblk}")
                                for hh in range(2)]
                        for hh in range(2):
                            nc.vector.tensor_copy(avst[hh][:], pos[hh][:])
                        base = avst
                    else:
                        base = pos
                    den0 = [ep.tile([1, SB], f32, tag=f"den{hh}", bufs=1,
                                    name=f"den{hh}_{p}_{qblk}")
                            for hh in range(2)]
                    rden = [ep.tile([1, SB], f32, tag=f"rden{hh}", bufs=1,
                                    name=f"rden{hh}_{p}_{qblk}")
                            for hh in range(2)]
                    rbc = [ep.tile([DH, SB], f32, tag=f"rbc{hh}", bufs=2,
                                   name=f"rbc{hh}_{p}_{qblk}")
                           for hh in range(2)]
                    for hh in range(2):
                        nc.vector.tensor_copy(den0[hh][:], base[hh][DH:DH + 1, :])
                    for hh in range(2):
                        nc.vector.reciprocal_approx_fast(
                            rden[hh][:], den0[hh][:])
                    for hh in range(2):
                        nc.gpsimd.partition_broadcast(
                            out_ap=rbc[hh][:], in_ap=rden[hh][:])
                    for hh in range(2):
                        nc.vector.tensor_mul(
                            O[p][hh * DH:hh * DH + DH,
                                 qblk * SB:(qblk + 1) * SB],
                            base[hh][0:DH, :],
                            rbc[hh][:],
                        )
                    # stage this (batch, quarter) into the sub-A2A buffer
                    nc.gpsimd.dma_start(
                        out=a2a_in[qblk][4 * p:4 * p + 4].rearrange(
                            "t p c -> p t c"),
                        in_=O[p][:, qblk * SB:(qblk + 1) * SB].rearrange(
                            "p (t c) -> p t c", t=4),
                    )
                if fire:
                    fire_collective(qblk)

            # ---------------------------------------------------------------
            def outproj(qblk, when=0.3):
              with tc.tile_wait_until(when):
                recv = []
                for m in range(NDC // 2):
                    rt = rp.tile([KC, 2 * KC], bf16, tag=f"rc{m}",
                                 name=f"rc{m}_{qblk}")
                    eng = nc.sync if m % 2 == 0 else nc.scalar
                    eng.dma_start(
                        out=rt[:].rearrange("p (t c) -> p t c", t=2),
                        in_=a2a_out[qblk][2 * m:2 * m + 2].rearrange(
                            "t p c -> p t c"),
                    )
                    recv.append(rt)
                for nb in range(D // SB):
                    ps = pp.tile([KC, SB], f32, tag="ps_qk", bufs=1)
                    for k in range(NDC):
                        nc.tensor.matmul(
                            ps[:],
                            recv[k // 2][:, (k % 2) * KC:(k % 2 + 1) * KC],
                            wout_t[:, k * D + nb * SB:k * D + (nb + 1) * SB],
                            start=(k == 0),
                            stop=(k == NDC - 1),
                        )
                    ot = ep.tile([KC, SB], f32, tag="osb", bufs=4)
                    nc.vector.tensor_add(
                        ot[:], ps[:], bo_bc_box[0][:, nb * SB:(nb + 1) * SB])
                    nc.sync.dma_start(
                        out=out_ext[qblk][:, nb * SB:(nb + 1) * SB],
                        in_=ot[:],
                    )

            # ---- static schedule ------------------------------------------
            proj(0)
            attention(0)
            proj(1)
            attention(1, fire=False)
            proj(2)
            attention(2, fire=False)
            fire_collective(1)
            proj(3)
            fire_collective(2)
            load_bo()
            attention(3, fire=False)
            outproj(0)
            outproj(1)
            outproj(2)
            fire_collective(3)
            outproj(3)

    nc.compile()
    return nc


def _get_program():
    global _compiled
    if _compiled is None:
        _compiled = _build()
    return _compiled


def _shard_inputs(x, Wqkv, bqkv, Wout, bout):
    """Build the 8 per-core input maps (all host-side numpy, bf16 data)."""
    bf = ml_dtypes.bfloat16
    x = np.asarray(x, dtype=np.float32)
    Wqkv = np.asarray(Wqkv, dtype=np.float32)
    bqkv = np.ascontiguousarray(np.asarray(bqkv, dtype=np.float32))
    Wout = np.asarray(Wout, dtype=np.float32)
    bout = np.ascontiguousarray(np.asarray(bout, dtype=np.float32))

    Wq = Wqkv[:, 0 * D:1 * D]
    Wk = Wqkv[:, 1 * D:2 * D]
    Wv_full = Wqkv[:, 2 * D:3 * D]
    bq = bqkv[0 * D:1 * D]
    bk = bqkv[1 * D:2 * D]
    bv_full = bqkv[2 * D:3 * D]

    # shared across all cores
    xt = np.ascontiguousarray(
        x.transpose(0, 2, 1)                      # [B, D, S]
         .reshape(B, D, NSB, SB).transpose(0, 2, 1, 3)
         .reshape(B, NSB, NDC, KC, SB).astype(bf)
    )
    wout_b = np.ascontiguousarray(Wout.reshape(NDC, KC, D).astype(bf))
    vones = np.ones((KC, NKC), dtype=bf)
    ident = np.eye(KC, dtype=bf)

    in_maps = []
    for c in range(NCORES):
        ha, hb = 2 * c, 2 * c + 1
        wqk_c = np.ascontiguousarray(np.concatenate(
            [Wq[:, ha * DH:(ha + 1) * DH], Wq[:, hb * DH:(hb + 1) * DH],
             Wk[:, ha * DH:(ha + 1) * DH], Wk[:, hb * DH:(hb + 1) * DH]],
            axis=1).reshape(NDC, KC, 2 * KC).astype(bf))
        bqk_c = np.ascontiguousarray(np.concatenate(
            [bq[ha * DH:(ha + 1) * DH], bq[hb * DH:(hb + 1) * DH],
             bk[ha * DH:(ha + 1) * DH], bk[hb * DH:(hb + 1) * DH]]))
        wv_c = np.ascontiguousarray(np.concatenate(
            [Wv_full[:, ha * DH:(ha + 1) * DH],
             Wv_full[:, hb * DH:(hb + 1) * DH]],
            axis=1).reshape(NDC, KC, KC).astype(bf))
        bv_c = np.ascontiguousarray(np.concatenate(
            [bv_full[ha * DH:(ha + 1) * DH], bv_full[hb * DH:(hb + 1) * DH]]))
        in_maps.append({
            "xt": xt, "wqk": wqk_c, "wv": wv_c, "wout": wout_b,
            "bqk": bqk_c, "bv": bv_c, "bo": bout, "vones": vones,
            "ident": ident,
        })
    return in_maps


def run(inputs, trace=False, trace_kwargs=None):
    nc = _get_program()
    in_maps = _shard_inputs(**inputs)
    res = run_bass_kernel_spmd(
        nc, in_maps, list(range(NCORES)), trace=trace,
        **(trace_kwargs or {}),
    )
    out = np.empty((B, S, D), dtype=np.float32)
    for c in range(NCORES):
        b = c // 4
        t4 = c % 4
        oc = res.results[c]["out"]  # [NSB, KC, D]
        for q in range(NSB):
            out[b, SB * q + KC * t4: SB * q + KC * (t4 + 1), :] = oc[q]
    return out, res


def kernel(**inputs):
    out, _ = run(inputs)
    return out


# revision 17
# speedup vs baseline: 1.0985x; 1.0985x over previous
"""Causal multi-head attention (B=2, S=2048, D=1024, H=16) on 8 trn2 cores.

Sharding: core c handles heads {2c, 2c+1} of BOTH batches (4 (b,h) pairs).
All matmul inputs are bf16 (host-rounded); accumulation stays fp32 in PSUM.

Per core:
  - project host-pretransposed x_b^T [D, S] (both batches) through the
    core's Wqkv column slice into Q^T/K^T head-pair tiles (bf16).  V is
    produced transposed (long moving dim), then flipped to natural layout
    with PE transposes; a fused ones-column makes AV emit softmax
    denominators,
  - causal attention per (batch, head-pair) in transposed layout:
    scores^T = K Q^T chunks as two row-tiled (tile_position) matmuls that
    stream concurrently, exp on ScalarE (bf16 out), diagonal masks via
    affine_select, A^T V accumulation on PE,
  - the head exchange is FOUR quarter-wise 8-way AllToAlls, fired as each
    sequence quarter finishes attention, so they overlap later attention.
    Sub-A2A q block t = (my heads, batch t//4, quarter q, col-slice t%4),
    so core i ends up owning tokens {512q + 128*(i%4)} of batch i//4 with
    ALL heads, and runs the output projection per received 128-token chunk.
    A dummy AllToAll issued at program start absorbs the ~11us collective
    firmware warmup.
Host assembles the 8x4 [128, 1024] shards into (2, 2048, 1024).
"""

import sys

for _p in ("/opt/trn_rl_repo", "/opt/pypackages"):
    if _p not in sys.path:
        sys.path.insert(0, _p)

import numpy as np
import ml_dtypes

import concourse.bass as bass
import concourse.mybir as mybir
import concourse.tile as tile
from concourse import bacc
from concourse.bass_utils import run_bass_kernel_spmd

B = 2
S = 2048
D = 1024
H = 16
DH = 64
NCORES = 8
SB = 512           # q block (matmul moving dim)
KC = 128           # k chunk (contraction tile)
NSB = S // SB      # 4 q-blocks
NKC = S // KC      # 16 k-chunks
NDC = D // KC      # 8 contraction chunks for the projections

_compiled = None


def _build():
    f32 = mybir.dt.float32
    bf16 = mybir.dt.bfloat16
    nc = bacc.Bacc(None, target_bir_lowering=False)

    # host-blocked inputs (bf16): xt[b, s, k] = x_b^T[128k:128k+128, 512s:+512]
    xt = nc.declare_dram_parameter("xt", [B, NSB, NDC, KC, SB], bf16, isOutput=False)
    # wqk cols: Q_ha | Q_hb | K_ha | K_hb (64 each)
    wqk = nc.declare_dram_parameter("wqk", [NDC, KC, 2 * KC], bf16, isOutput=False)
    # wv cols: V_ha | V_hb
    wv = nc.declare_dram_parameter("wv", [NDC, KC, KC], bf16, isOutput=False)
    wout = nc.declare_dram_parameter("wout", [NDC, KC, D], bf16, isOutput=False)
    bqk = nc.declare_dram_parameter("bqk", [2 * KC], f32, isOutput=False)
    bv = nc.declare_dram_parameter("bv", [2 * DH], f32, isOutput=False)
    bo = nc.declare_dram_parameter("bo", [D], f32, isOutput=False)
    vones = nc.declare_dram_parameter("vones", [KC, NKC], bf16, isOutput=False)
    ident = nc.declare_dram_parameter("ident", [KC, KC], bf16, isOutput=False)
    # out[q] = final rows for tokens [512q + 128*(c%4), +128) of batch c//4
    out_ext = nc.declare_dram_parameter("out", [NSB, KC, D], f32, isOutput=True)

    # quarter-wise AllToAll staging: sub-A2A q block t =
    #   (my 128 head rows, batch t//4, quarter q, col-slice 128*(t%4))
    a2a_in = [nc.dram_tensor(f"a2a_in{q}", [NCORES, KC, KC], bf16)
              for q in range(NSB)]
    a2a_out = [nc.dram_tensor(f"a2a_out{q}", [NCORES, KC, KC], bf16)
               for q in range(NSB)]

    with tile.TileContext(nc) as tc:
        with (
            tc.tile_pool(name="qkv", bufs=1) as qkvp,
            tc.tile_pool(name="obuf", bufs=1) as op,
            tc.tile_pool(name="misc", bufs=1) as mp,
            tc.tile_pool(name="pbuf", bufs=1) as pb,
            tc.tile_pool(name="evict", bufs=1) as ep,
            tc.tile_pool(name="wpool", bufs=1) as wp,
            tc.tile_pool(name="xbuf", bufs=10) as xp,
            tc.tile_pool(name="recvp", bufs=1) as rp,
            tc.tile_pool(name="psum", bufs=1, space="PSUM") as pp,
        ):
            # ---- weights + small constants --------------------------------
            # sync queue: wqk first (needed by the very first matmul), then
            # the first x tiles.  Big wout load rides the vector queue.
            wqk_t = wp.tile([KC, NDC * 2 * KC], bf16, tag="wqk")
            nc.sync.dma_start(
                out=wqk_t[:].rearrange("p (k c) -> p k c", k=NDC),
                in_=wqk.rearrange("k p c -> p k c"))
            vones_first = True
            bqk_t = [mp.tile([KC, 1], f32, tag=f"bqk{m}", name=f"bqk{m}")
                     for m in range(2)]
            for m in range(2):
                nc.scalar.dma_start(
                    out=bqk_t[m][:],
                    in_=bqk[m * KC:(m + 1) * KC].rearrange("(p o) -> p o", o=1),
                )
            bv_t = mp.tile([KC, 1], f32, tag="bv_t")
            nc.scalar.dma_start(
                out=bv_t[:], in_=bv.rearrange("(p o) -> p o", o=1))
            ident_t = mp.tile([KC, KC], bf16, tag="ident")
            nc.scalar.dma_start(out=ident_t[:], in_=ident[:])
            vones_sb = mp.tile([KC, NKC], bf16, tag="vones_sb")
            nc.scalar.dma_start(out=vones_sb[:], in_=vones[:])
            act_warm = mp.tile([1, 2], f32, tag="act_warm")
            nc.scalar.activation(
                act_warm[:], vones_sb[0:1, 0:2],
                mybir.ActivationFunctionType.Exp, scale=1.0)
            wv_t = wp.tile([KC, NDC * KC], bf16, tag="wv")
            nc.scalar.dma_start(
                out=wv_t[:].rearrange("p (k c) -> p k c", k=NDC),
                in_=wv.rearrange("k p c -> p k c"))
            wout_t = wp.tile([KC, NDC * D], bf16, tag="wout")
            bo_bc_box = []

            def load_bo():
                nc.sync.dma_start(
                    out=wout_t[:].rearrange("p (k c) -> p k c", k=NDC),
                    in_=wout.rearrange("k p c -> p k c"))
                bo_row = mp.tile([1, D], f32, tag="bo_row")
                nc.scalar.dma_start(
                    out=bo_row[:], in_=bo.rearrange("(o f) -> o f", o=1))
                t = mp.tile([KC, D], f32, tag="bo_bc")
                nc.gpsimd.partition_broadcast(out_ap=t[:], in_ap=bo_row[:])
                bo_bc_box.append(t)

            # ---- persistent activations -----------------------------------
            # QQ[p][s]: rows 0:64 = Q^T head 2c, 64:128 = head 2c+1 (batch p)
            QQ = [[qkvp.tile([KC, SB], bf16, tag=f"QQ{p}_{s}", name=f"QQ{p}_{s}")
                   for s in range(NSB)] for p in range(B)]
            KK = [[qkvp.tile([KC, SB], bf16, tag=f"KK{p}_{s}", name=f"KK{p}_{s}")
                   for s in range(NSB)] for p in range(B)]
            # V[2p+hh][s]: [128, 4*65]; chunk sc at cols 65sc..+64, col 65sc+64=1
            NCS = SB // KC
            V = [[qkvp.tile([KC, NCS * (DH + 1)], bf16, tag=f"V{v}_{s}",
                            name=f"V{v}_{s}")
                  for s in range(NSB)] for v in range(2 * B)]
            # O[p]: rows 0:64 = head 2c out^T (normalized), 64:128 = head 2c+1
            O = [op.tile([KC, S], bf16, tag=f"O{p}", name=f"O{p}")
                 for p in range(B)]

            # P score tiles (bf16, post-exp).  Tag per k-chunk; low chunks
            # are double-buffered since they recur every quarter.
            def p_tile(p, q, kc):
                return pb.tile([KC, 2, SB], bf16, tag=f"P{kc}",
                               name=f"P{p}_{q}_{kc}",
                               bufs=(2 if kc < 8 else 1))

            # ---------------------------------------------------------------
            def proj(sblk):
                vts = []
                for bb in range(B):
                    xs = []
                    for j in range(NDC // 2):
                        xtl = xp.tile([KC, 2 * SB], bf16, tag="xt")
                        if sblk == 0:
                            eng = (nc.sync, nc.scalar, nc.sync, nc.scalar)[j]
                        else:
                            eng = (nc.sync, nc.gpsimd, nc.sync, nc.gpsimd)[j]
                        eng.dma_start(
                            out=xtl[:].rearrange("p (k t) -> p k t", k=2),
                            in_=xt[bb, sblk, 2 * j:2 * j + 2].rearrange(
                                "k p t -> p k t"),
                        )
                        xs.append(xtl)

                    def xchunk(k):
                        return xs[k // 2][:, (k % 2) * SB:(k % 2 + 1) * SB]

                    # m = 0 -> Q^T pair, m = 1 -> K^T pair
                    for m in range(2):
                        ps = pp.tile([KC, SB], f32, tag="ps_qk", bufs=1)
                        for k in range(NDC):
                            nc.tensor.matmul(
                                ps[:],
                                wqk_t[:, (2 * k + m) * KC:(2 * k + m + 1) * KC],
                                xchunk(k),
                                start=(k == 0),
                                stop=(k == NDC - 1),
                            )
                        dest = (QQ if m == 0 else KK)[bb][sblk]
                        nc.vector.tensor_scalar_add(dest[:], ps[:], bqk_t[m][:])
                    # V^T: long moving dim, then flip via PE transposes below
                    ps = pp.tile([KC, SB], f32, tag="ps_qk", bufs=1)
                    for k in range(NDC):
                        nc.tensor.matmul(
                            ps[:],
                            wv_t[:, k * KC:(k + 1) * KC],
                            xchunk(k),
                            start=(k == 0),
                            stop=(k == NDC - 1),
                        )
                    vt = ep.tile([KC, SB], bf16, tag="vt", bufs=2,
                                 name=f"vt{bb}_{sblk}")
                    nc.vector.tensor_scalar_add(vt[:], ps[:], bv_t[:])
                    vts.append(vt)
                for bb in range(B):
                    pst = pp.tile([KC, SB], bf16, tag="ps_tr", bufs=1)
                    for sc in range(NCS):
                        nc.tensor.transpose(
                            pst[:, sc * KC:(sc + 1) * KC],
                            vts[bb][:, sc * KC:(sc + 1) * KC],
                            ident_t[:],
                        )
                    ps4 = pst[:].rearrange("p (k h c) -> p k h c", k=NCS, h=2)
                    for hh in range(2):
                        nc.vector.tensor_copy(
                            V[2 * bb + hh][sblk][:].rearrange(
                                "p (k c) -> p k c", c=DH + 1)[:, :, 0:DH],
                            ps4[:, :, hh, :],
                        )
                        vv = V[2 * bb + hh][sblk][:].rearrange(
                            "p (k c) -> p k c", c=DH + 1)
                        nc.vector.tensor_copy(
                            vv[:, :, DH],
                            vones_sb[:, sblk * NCS:(sblk + 1) * NCS])

            # ---------------------------------------------------------------
            def fire_collective(qblk):
                nc.gpsimd.collective_compute(
                    "AllToAll",
                    mybir.AluOpType.bypass,
                    replica_groups=[[0, 1, 2, 3, 4, 5, 6, 7]],
                    ins=[a2a_in[qblk][:]],
                    outs=[a2a_out[qblk][:]],
                )

            def attention(qblk, fire=True):
                nkc = 4 * (qblk + 1)
                for p in range(B):
                    pos = [pp.tile([DH + 1, SB], f32, tag=f"ps_av{hh}",
                                   bufs=1, name=f"po{hh}_{p}_{qblk}")
                           for hh in range(2)]
                    P = [None] * nkc

                    def scores(kc):
                        d = kc - 4 * qblk
                        c0 = KC * max(d, 0)
                        ps = pp.tile([KC, 2, SB], f32, tag="ps_s", bufs=2)
                        for hh in range(2):  # row-tiled, stream concurrently
                            r0 = hh * DH
                            nc.tensor.matmul(
                                ps[:, hh, c0:SB],
                                KK[p][kc // 4][r0:r0 + DH,
                                               (kc % 4) * KC:(kc % 4 + 1) * KC],
                                QQ[p][qblk][r0:r0 + DH, c0:SB],
                                start=True,
                                stop=True,
                            )
                        P[kc] = p_tile(p, qblk, kc)
                        nc.scalar.activation(
                            P[kc][:, :, c0:SB],
                            ps[:, :, c0:SB],
                            mybir.ActivationFunctionType.Exp,
                            scale=1.0 / float(np.sqrt(DH)),
                        )
                        if d >= 0:  # diagonal chunk: zero where k > q
                            nc.gpsimd.affine_select(
                                out=P[kc][:, :, c0:SB],
                                in_=P[kc][:, :, c0:SB],
                                pattern=[[0, 2], [1, SB - c0]],
                                compare_op=mybir.AluOpType.is_ge,
                                fill=0.0,
                                base=0,
                                channel_multiplier=-1,
                            )

                    def av(kc):
                        d = kc - 4 * qblk
                        c0 = KC * max(d, 0)
                        for hh in range(2):
                            nc.tensor.matmul(
                                pos[hh][:, c0:SB],
                                V[2 * p + hh][kc // 4][:,
                                    (kc % 4) * (DH + 1):
                                    (kc % 4 + 1) * (DH + 1)],
                                P[kc][:, hh, c0:SB],
                                start=(kc == 0),
                                stop=(kc == nkc - 1),
                            )

                    # interleave: sc(kc) | av(kc-1) keeps ScalarE saturated
                    for kc in range(nkc):
                        scores(kc)
                        if kc >= 1:
                            av(kc - 1)
                    av(nkc - 1)

                    # normalize (PSUM reads stay on DVE; broadcast on GpSimd).
                    # For quarters 0-2, evacuate pos to SBUF in one copy so
                    # the next batch's AV can reuse the PSUM bank at once;
                    # for the tail quarter read PSUM directly (shorter chain).
                    if qblk < 3:
                        avst = [ep.tile([DH + 1, SB], f32, tag=f"avst{hh}",
                                        bufs=2, name=f"avst{hh}_{p}_{qblk}")
                                for hh in range(2)]
                        for hh in range(2):
                            nc.vector.tensor_copy(avst[hh][:], pos[hh][:])
                        base = avst
                    else:
                        base = pos
                    den0 = [ep.tile([1, SB], f32, tag=f"den{hh}", bufs=1,
                                    name=f"den{hh}_{p}_{qblk}")
                            for hh in range(2)]
                    rden = [ep.tile([1, SB], f32, tag=f"rden{hh}", bufs=1,
                                    name=f"rden{hh}_{p}_{qblk}")
                            for hh in range(2)]
                    rbc = [ep.tile([DH, SB], f32, tag=f"rbc{hh}", bufs=2,
                                   name=f"rbc{hh}_{p}_{qblk}")
                           for hh in range(2)]
                    for hh in range(2):
                        nc.vector.tensor_copy(den0[hh][:], base[hh][DH:DH + 1, :])
                    for hh in range(2):
                        nc.vector.reciprocal_approx_fast(
                            rden[hh][:], den0[hh][:])
                    for hh in range(2):
                        nc.gpsimd.partition_broadcast(
                            out_ap=rbc[hh][:], in_ap=rden[hh][:])
                    for hh in range(2):
                        nc.vector.tensor_mul(
                            O[p][hh * DH:hh * DH + DH,
                                 qblk * SB:(qblk + 1) * SB],
                            base[hh][0:DH, :],
                            rbc[hh][:],
                        )
                    # stage this (batch, quarter) into the sub-A2A buffer
                    nc.gpsimd.dma_start(
                        out=a2a_in[qblk][4 * p:4 * p + 4].rearrange(
                            "t p c -> p t c"),
                        in_=O[p][:, qblk * SB:(qblk + 1) * SB].rearrange(
                            "p (t c) -> p t c", t=4),
                    )
                if fire:
                    fire_collective(qblk)

            # ---------------------------------------------------------------
            def outproj(qblk, when=0.3):
              with tc.tile_wait_until(when):
                recv = []
                for m in range(NDC // 2):
                    rt = rp.tile([KC, 2 * KC], bf16, tag=f"rc{m}",
                                 name=f"rc{m}_{qblk}")
                    eng = nc.sync if m % 2 == 0 else nc.scalar
                    eng.dma_start(
                        out=rt[:].rearrange("p (t c) -> p t c", t=2),
                        in_=a2a_out[qblk][2 * m:2 * m + 2].rearrange(
                            "t p c -> p t c"),
                    )
                    recv.append(rt)
                for nb in range(D // SB):
                    ps = pp.tile([KC, SB], f32, tag="ps_qk", bufs=1)
                    for k in range(NDC):
                        nc.tensor.matmul(
                            ps[:],
                            recv[k // 2][:, (k % 2) * KC:(k % 2 + 1) * KC],
                            wout_t[:, k * D + nb * SB:k * D + (nb + 1) * SB],
                            start=(k == 0),
                            stop=(k == NDC - 1),
                        )
                    ot = ep.tile([KC, SB], f32, tag="osb", bufs=4)
                    nc.vector.tensor_add(
                        ot[:], ps[:], bo_bc_box[0][:, nb * SB:(nb + 1) * SB])
                    nc.sync.dma_start(
                        out=out_ext[qblk][:, nb * SB:(nb + 1) * SB],
                        in_=ot[:],
                    )

            # ---- static schedule ------------------------------------------
            proj(0)
            attention(0)
            proj(1)
            attention(1, fire=False)
            proj(2)
            attention(2, fire=False)
            fire_collective(1)
            proj(3)
            fire_collective(2)
            load_bo()
            attention(3, fire=False)
            outproj(0)
            outproj(1)
            outproj(2)
            fire_collective(3)
            outproj(3)

    nc.compile()
    return nc


def _get_program():
    global _compiled
    if _compiled is None:
        _compiled = _build()
    return _compiled


def _shard_inputs(x, Wqkv, bqkv, Wout, bout):
    """Build the 8 per-core input maps (all host-side numpy, bf16 data)."""
    bf = ml_dtypes.bfloat16
    x = np.asarray(x, dtype=np.float32)
    Wqkv = np.asarray(Wqkv, dtype=np.float32)
    bqkv = np.ascontiguousarray(np.asarray(bqkv, dtype=np.float32))
    Wout = np.asarray(Wout, dtype=np.float32)
    bout = np.ascontiguousarray(np.asarray(bout, dtype=np.float32))

    Wq = Wqkv[:, 0 * D:1 * D]
    Wk = Wqkv[:, 1 * D:2 * D]
    Wv_full = Wqkv[:, 2 * D:3 * D]
    bq = bqkv[0 * D:1 * D]
    bk = bqkv[1 * D:2 * D]
    bv_full = bqkv[2 * D:3 * D]

    # shared across all cores
    xt = np.ascontiguousarray(
        x.transpose(0, 2, 1)                      # [B, D, S]
         .reshape(B, D, NSB, SB).transpose(0, 2, 1, 3)
         .reshape(B, NSB, NDC, KC, SB).astype(bf)
    )
    wout_b = np.ascontiguousarray(Wout.reshape(NDC, KC, D).astype(bf))
    vones = np.ones((KC, NKC), dtype=bf)
    ident = np.eye(KC, dtype=bf)

    in_maps = []
    for c in range(NCORES):
        ha, hb = 2 * c, 2 * c + 1
        wqk_c = np.ascontiguousarray(np.concatenate(
            [Wq[:, ha * DH:(ha + 1) * DH], Wq[:, hb * DH:(hb + 1) * DH],
             Wk[:, ha * DH:(ha + 1) * DH], Wk[:, hb * DH:(hb + 1) * DH]],
            axis=1).reshape(NDC, KC, 2 * KC).astype(bf))
        bqk_c = np.ascontiguousarray(np.concatenate(
            [bq[ha * DH:(ha + 1) * DH], bq[hb * DH:(hb + 1) * DH],
             bk[ha * DH:(ha + 1) * DH], bk[hb * DH:(hb + 1) * DH]]))
        wv_c = np.ascontiguousarray(np.concatenate(
            [Wv_full[:, ha * DH:(ha + 1) * DH],
             Wv_full[:, hb * DH:(hb + 1) * DH]],
            axis=1).reshape(NDC, KC, KC).astype(bf))
        bv_c = np.ascontiguousarray(np.concatenate(
            [bv_full[ha * DH:(ha + 1) * DH], bv_full[hb * DH:(hb + 1) * DH]]))
        in_maps.append({
            "xt": xt, "wqk": wqk_c, "wv": wv_c, "wout": wout_b,
            "bqk": bqk_c, "bv": bv_c, "bo": bout, "vones": vones,
            "ident": ident,
        })
    return in_maps


def run(inputs, trace=False, trace_kwargs=None):
    nc = _get_program()
    in_maps = _shard_inputs(**inputs)
    res = run_bass_kernel_spmd(
        nc, in_maps, list(range(NCORES)), trace=trace,
        **(trace_kwargs or {}),
    )
    out = np.empty((B, S, D), dtype=np.float32)
    for c in range(NCORES):
        b = c // 4
        t4 = c % 4
        oc = res.results[c]["out"]  # [NSB, KC, D]
        for q in range(NSB):
            out[b, SB * q + KC * t4: SB * q + KC * (t4 + 1), :] = oc[q]
    return out, res


def kernel(**inputs):
    out, _ = run(inputs)
    return out
